# revision 15
# baseline (speedup 1.0000x reference)
"""Trainium2 Bass kernel for nn_MiM_v2 (Mamba-style selective scan).

Sharding: pure data-parallel over batch B=16 across 8 NeuronCores
(2 batches per core, weights replicated, no collectives).

Per-core pipeline over token chunks (TC tokens at a time):
  fused in_proj+causal-conv: 3 shifted bf16 matmuls with conv-prescaled
    weights + folded bias, x loaded with a 2-token halo (PE only)
  -> silu (ACT), fused across dt-pairs (one (128,1024) op per pair)
  -> x_proj (PE) -> grouped RMSNorm (PE reduce + ACT ln/exp + DVE)
  -> dt_proj (PE, pair-fused psum) -> softplus as exp+ln pairs; both
     functions resolve to act table set 6 (natural_log_exp_and_others),
     pinned by one explicit LoadActFuncSet so no table thrash
  -> deltaA = exp(A*delta): A[d,n] = -(n+1) is constant across d, so
     exps fuse across dt-pairs (one (128, 2*TC) ACT op per (pair,qf,n))
  -> BX = delta*h*B (DVE, broadcast AP vs DRAM-bounced B/C broadcast)
  -> linear recurrence via DVE tensor_tensor_scan; cross-chunk carries
     via per-n init columns folded into bx col 0
  -> y = sum_n hs*C (DVE mult; tree-reduce levels on GpSimd)
  -> out_proj (bf16 PE) -> DMA out
"""

import sys

if "/opt/trn_rl_repo" not in sys.path:
    sys.path.insert(0, "/opt/trn_rl_repo")

import numpy as np
import ml_dtypes

import concourse.bass as bass
import concourse.mybir as mybir
import concourse.tile as tile
from concourse import bacc

# ---------------------------------------------------------------- constants
B, L, DM = 16, 1024, 512
DIN, DT, N, K = 2 * DM, 32, 16, 3
NCORES = 8
BPC = B // NCORES          # batches per core
T = BPC * L                # tokens per core
TC = 512                   # token chunk
NCH = T // TC              # chunks per core
CPB = L // TC              # chunks per batch
NDT = DIN // 128           # d-inner tiles
NPAIR = NDT // 2           # dt pairs
NKT = DM // 128            # k tiles for in_proj
NH = 2                     # n-groups for the B/C broadcast layout
NPH = N // NH              # n per half (8)
NQ = 4                     # n quarter-groups for scan/readout
NPQ = N // NQ              # n per quarter (4)
TBLK = TC // 128           # 128-token blocks per chunk

F32 = mybir.dt.float32
F32R = mybir.dt.float32r
BF16 = mybir.dt.bfloat16
AF = mybir.ActivationFunctionType
ALU = mybir.AluOpType

FOLD_D_HOST = False        # D_skip applied as STT on DVE (saves SBUF + PE)
MMDT = F32R
ACT_SET_LN_EXP = 6         # natural_log_exp_and_others

GP_TREE = False            # gpsimd TT measured ~2x worse than spec; keep tree on DVE

# set per-input-data in kernel(): exps fuse across dt only when A rows
# are identical (A_log = log(tile(arange)) in the reference)
A_UNIFORM = True


def _r(ap):
    return ap


# ---------------------------------------------------------------- host prep
def host_weights(inp):
    """Precompute transposed/reorganized weights (numpy, shared by all cores)."""
    f = lambda x: np.ascontiguousarray(np.asarray(x, np.float32))
    w = {}
    cw = np.asarray(inp["conv_w"], np.float32)[:, 0, :]   # (DIN, 3)
    in_wT = np.asarray(inp["in_w"], np.float32).T          # (DM, DIN)
    for k in range(K):
        w[f"wk{k}"] = np.ascontiguousarray(
            (0.5 * in_wT * cw[None, :, k]).astype(ml_dtypes.bfloat16))
    in_b = np.asarray(inp["in_b"], np.float32)
    conv_b = np.asarray(inp["conv_b"], np.float32)
    bf = lambda x: np.ascontiguousarray(np.asarray(x).astype(ml_dtypes.bfloat16))
    w["b_eff"] = bf(0.5 * (in_b * cw.sum(1) + conv_b))[None, :]      # (1, DIN)
    w["bfix"] = bf(np.stack([-0.5 * in_b * (cw[:, 0] + cw[:, 1]),
                             -0.5 * in_b * cw[:, 0]]))              # (2, DIN)
    w["eye2"] = bf(np.eye(2, dtype=np.float32))
    w["w_x_T"] = np.ascontiguousarray(
        np.asarray(inp["xproj_w"], np.float32).T.astype(ml_dtypes.bfloat16))
    w["w_dt_T"] = np.ascontiguousarray(
        np.asarray(inp["dt_w"], np.float32).T.astype(ml_dtypes.bfloat16))
    w["w_out_T"] = np.ascontiguousarray(
        np.asarray(inp["out_w"], np.float32).T.astype(ml_dtypes.bfloat16))
    if FOLD_D_HOST:
        w["w_out_D"] = np.ascontiguousarray(
            (np.asarray(inp["D_skip"], np.float32)[:, None]
             * np.asarray(inp["out_w"], np.float32).T
             ).astype(ml_dtypes.bfloat16))
    w["A_neg"] = f(-np.exp(np.asarray(inp["A_log"], np.float64)))  # (DIN, N)
    w["b_dt"] = bf(inp["dt_b"][None, :])                # (1, DIN)
    w["b_out"] = bf(inp["out_b"][None, :])              # (1, DM)
    w["d_col"] = f(np.asarray(inp["D_skip"])[:, None])  # (DIN, 1)
    w["lnw"] = f(np.concatenate(
        [inp["dtln_w"], inp["Bln_w"], inp["Cln_w"]])[:, None])  # (64, 1)
    m_ms = np.zeros((DT + 2 * N, 3), np.float32)
    m_ms[:DT, 0] = 1.0 / DT
    m_ms[DT:DT + N, 1] = 1.0 / N
    m_ms[DT + N:, 2] = 1.0 / N
    w["m_ms"] = m_ms.astype(ml_dtypes.bfloat16)
    e_bc = np.zeros((3, DT + 2 * N), np.float32)
    e_bc[0, :DT] = 1.0
    e_bc[1, DT:DT + N] = 1.0
    e_bc[2, DT + N:] = 1.0
    w["e_bc"] = e_bc
    w["ones_row"] = np.ones((1, TC), ml_dtypes.bfloat16)
    w["i128"] = np.ascontiguousarray(np.eye(128, dtype=ml_dtypes.bfloat16))
    return w


def host_x_shard(x, core):
    """x (B, L, DM) -> per-core transposed bf16 shard (BPC, DM, L)."""
    xs = np.asarray(x, np.float32)[core * BPC:(core + 1) * BPC]
    return np.ascontiguousarray(
        xs.transpose(0, 2, 1).astype(ml_dtypes.bfloat16))


# ---------------------------------------------------------------- IO decl
def declare_ios(nc):
    def d(name, shape=None, dt=F32):
        return nc.dram_tensor(name, list(shape), dt,
                              kind="ExternalInput").ap()
    ins = {
        "xT": d("xT", dt=BF16, shape=(BPC, DM, L)),
        "wk0": d("wk0", dt=BF16, shape=(DM, DIN)),
        "wk1": d("wk1", dt=BF16, shape=(DM, DIN)),
        "wk2": d("wk2", dt=BF16, shape=(DM, DIN)),
        "b_eff": d("b_eff", dt=BF16, shape=(1, DIN)),
        "bfix": d("bfix", dt=BF16, shape=(2, DIN)),
        "eye2": d("eye2", dt=BF16, shape=(2, 2)),
        "w_x_T": d("w_x_T", dt=BF16, shape=(DIN, DT + 2 * N)),
        "w_dt_T": d("w_dt_T", dt=BF16, shape=(DT, DIN)),
        "w_out_T": d("w_out_T", dt=BF16, shape=(DIN, DM)),
        "A_neg": d("A_neg", (DIN, N)),
        "b_dt": d("b_dt", dt=BF16, shape=(1, DIN)),
        "b_out": d("b_out", dt=BF16, shape=(1, DM)),
        "d_col": d("d_col", (DIN, 1)),
        "lnw": d("lnw", (DT + 2 * N, 1)),
        "m_ms": d("m_ms", dt=BF16, shape=(DT + 2 * N, 3)),
        "e_bc": d("e_bc", dt=MMDT, shape=(3, DT + 2 * N)),
        "ones_row": d("ones_row", dt=BF16, shape=(1, TC)),
        "i128": d("i128", dt=BF16, shape=(128, 128)),
    }
    if FOLD_D_HOST:
        ins["w_out_D"] = d("w_out_D", (DIN, DM), dt=BF16)
    outs = {
        "y_out": nc.dram_tensor("y_out", [BPC, L, DM], F32,
                                kind="ExternalOutput").ap(),
    }
    return ins, outs


def _load_act_set(nc, set_id, dep_aps=()):
    """Explicit act-table load; the table-load pass then treats every
    function present in this set as already loaded on paths after it."""
    ins = [nc.scalar.lower_ap(ap) for ap in dep_aps]
    nc.scalar.add_instruction(mybir.InstLoadActFuncSet(
        name=nc.get_next_instruction_name(),
        act_func_set_id=set_id, ins=ins, outs=[]))


# ---------------------------------------------------------------- kernel body
def emit(tc_ctx, outs, ins):
    from contextlib import ExitStack
    tc = tc_ctx
    nc = tc.nc
    G = DT + 2 * N  # 64
    TC2 = 2 * TC

    st = ExitStack()
    pool = lambda **kw: st.enter_context(tc.tile_pool(**kw))
    cpool = pool(name="consts", bufs=1)
    xpool = pool(name="xck", bufs=1)
    hpool = pool(name="h", bufs=2)
    trpool = pool(name="transient", bufs=2)
    spool = pool(name="smalls", bufs=1)
    esppool = pool(name="esp", bufs=2)
    dltpool = pool(name="dlt", bufs=2)
    upool = pool(name="u", bufs=2)
    dApool = pool(name="dA", bufs=2)
    hspool = pool(name="hs", bufs=2)
    bxpool = pool(name="bx", bufs=2)
    bcpool = pool(name="bcb", bufs=1)
    ypool = pool(name="y", bufs=1)
    opool = pool(name="osb", bufs=1)
    hstpool = pool(name="hsT", bufs=2)
    dgpool = pool(name="diag", bufs=1)
    ytpool = pool(name="ysb", bufs=2)
    bctpool = pool(name="bcT", bufs=2)
    pp_h = pool(name="ph", bufs=1, space="PSUM")
    pp_misc = pool(name="pmisc", bufs=1, space="PSUM")
    pp_d = pool(name="pd", bufs=1, space="PSUM")
    pp_o = pool(name="po", bufs=1, space="PSUM")
    pp_y = pool(name="py", bufs=1, space="PSUM")

    dma = nc.sync.dma_start

    # ---- persistent constants -------------------------------------------
    def const_tile(name, shape=None, src=None, dt=F32):
        t = cpool.tile(list(shape), dt, tag=name)
        if src.dtype != dt and mybir.dt.size(src.dtype) == mybir.dt.size(dt):
            src = src.bitcast(dt)
        dma(t[:], src)
        return t

    wk_sb = [[const_tile(f"wk{k}_{kt}", (128, DIN),
                         ins[f"wk{k}"][kt * 128:(kt + 1) * 128, :], dt=BF16)
              for kt in range(NKT)] for k in range(K)]
    xproj_wT = [const_tile(f"xp_wT{k}", (128, G),
                           ins["w_x_T"][k * 128:(k + 1) * 128, :], dt=BF16)
                for k in range(NDT)]
    dt_wT = const_tile("dt_wT", (DT, DIN), ins["w_dt_T"][:, :], dt=BF16)
    out_wT = [const_tile(f"out_wT{k}", (128, DM),
                         ins["w_out_T"][k * 128:(k + 1) * 128, :], dt=BF16)
              for k in range(NDT)]
    if FOLD_D_HOST:
        out_wD = [const_tile(f"out_wD{k}", (128, DM),
                             ins["w_out_D"][k * 128:(k + 1) * 128, :],
                             dt=BF16)
                  for k in range(NDT)]
    A_sb = [const_tile(f"A{k}", (128, N),
                       ins["A_neg"][k * 128:(k + 1) * 128, :])
            for k in range(NDT)]
    d_col = [const_tile(f"D{k}", (128, 1),
                        ins["d_col"][k * 128:(k + 1) * 128, :])
             for k in range(NDT)]
    b_eff = const_tile("b_eff", dt=BF16, shape=(1, DIN), src=ins["b_eff"][:, :])
    bfix = const_tile("bfix", dt=BF16, shape=(2, DIN), src=ins["bfix"][:, :])
    eye2 = const_tile("eye2", dt=BF16, shape=(2, 2), src=ins["eye2"][:, :])
    b_dt = const_tile("b_dt", dt=BF16, shape=(1, DIN), src=ins["b_dt"][:, :])
    b_out = const_tile("b_out", dt=BF16, shape=(1, DM), src=ins["b_out"][:, :])
    lnw = const_tile("lnw", (G, 1), ins["lnw"][:, :])
    m_ms = const_tile("m_ms", (G, 3), ins["m_ms"][:, :], dt=BF16)
    e_bc = const_tile("e_bc", (3, G), ins["e_bc"][:, :], dt=MMDT)
    ones = const_tile("ones", (1, TC), ins["ones_row"][:, :], dt=BF16)

    eps = cpool.tile([128, 1], F32, tag="eps")
    nc.vector.memset(eps[:], 1e-5)
    i_sb = const_tile("i128", (128, 128), ins["i128"][:, :], dt=BF16)

    # persistent cross-chunk state
    state = cpool.tile([128, NDT * N], F32, tag="state")      # scan carries

    # DRAM bounce buffer for the B/C broadcast
    bc_dram = nc.dram_tensor("bc_scratch", [NCH, NH, 2 * NPH, TC], BF16).ap()

    # pin the ln+exp table set once at program start
    _load_act_set(nc, ACT_SET_LN_EXP)

    gp_add = nc.gpsimd.tensor_add if GP_TREE else nc.vector.tensor_add

    for ch in range(NCH):
        bb, cb = divmod(ch, CPB)

        # ---- load x chunk with 2-col halo (already transposed on host) ---
        xck = []
        for kt in range(NKT):
            t = xpool.tile([128, TC + 2], BF16, tag=f"x{kt}")
            if cb == 0:
                nc.vector.memset(t[:, 0:2], 0.0)
                dma(t[:, 2:TC + 2], ins["xT"][bb, kt * 128:(kt + 1) * 128,
                                              0:TC])
            else:
                dma(t[:], ins["xT"][bb, kt * 128:(kt + 1) * 128,
                                    cb * TC - 2:cb * TC + TC])
            xck.append(t)

        # ---- stage A: fused in_proj+conv (PE) -> silu, dt-pair fused -----
        h_all = hpool.tile([128, NDT * TC], BF16, tag="hall")
        h_list = [h_all[:, dt * TC:(dt + 1) * TC] for dt in range(NDT)]
        for dt in range(NDT):
            ph = pp_h.tile([128, TC], F32, tag="ph")
            ds = slice(dt * 128, (dt + 1) * 128)
            first = True
            for k in range(K):
                for kt in range(NKT):
                    nc.tensor.matmul(
                        ph[:], _r(wk_sb[k][kt][:, ds]),
                        _r(xck[kt][:, k:k + TC]), start=first, stop=False)
                    first = False
            if cb == 0:
                nc.tensor.matmul(ph[:, 0:2], _r(bfix[:, ds]),
                                 _r(eye2[:]), start=False, stop=False)
            nc.tensor.matmul(ph[:], _r(b_eff[0:1, ds]),
                             _r(ones[0:1, 0:TC]), start=False, stop=True)
            # h = silu(2*ph) (scale undoes the 0.5 prescale)
            nc.scalar.activation(h_list[dt], ph[:], AF.Silu, scale=2.0)
        # restore ln/exp tables after the silu batch
        _load_act_set(nc, ACT_SET_LN_EXP,
                      dep_aps=[h_all[:, (NDT - 1) * TC:NDT * TC]])

        # ---- stage C: x_proj + grouped rmsnorm ---------------------------
        pdbc = pp_misc.tile([G, TC], F32, tag="pmisc")
        for kt in range(NDT):
            nc.tensor.matmul(pdbc[:], _r(xproj_wT[kt][:]), _r(h_list[kt]),
                             start=(kt == 0), stop=(kt == NDT - 1))
        dbc_sb = spool.tile([G, TC], F32, tag="dbc")
        nc.scalar.copy(dbc_sb[:], pdbc[:])
        sq = spool.tile([G, TC], BF16, tag="sq")
        nc.scalar.activation(sq[:], pdbc[:], AF.Square)
        pms = pp_misc.tile([3, TC], F32, tag="pmisc")
        nc.tensor.matmul(pms[:], _r(m_ms[:]), _r(sq[:]), start=True, stop=True)
        lnm = spool.tile([3, TC], F32, tag="lnm")
        nc.scalar.activation(lnm[:], pms[:], AF.Ln, bias=eps[0:3, :])
        rin = spool.tile([3, TC], MMDT, tag="rin")
        nc.scalar.activation(rin[:], lnm[:], AF.Exp, scale=-0.5)
        pr = pp_misc.tile([G, TC], F32, tag="pmisc")
        nc.tensor.matmul(pr[:], _r(e_bc[:]), _r(rin[:]), start=True, stop=True)
        delta_n = spool.tile([DT, TC], BF16, tag="dn")
        nc.vector.scalar_tensor_tensor(
            delta_n[:], dbc_sb[0:DT, :], lnw[0:DT, :], pr[0:DT, :],
            op0=ALU.mult, op1=ALU.mult)
        bc_n = spool.tile([2 * N, TC], BF16, tag="bcn")
        nc.vector.scalar_tensor_tensor(
            bc_n[:], dbc_sb[DT:G, :], lnw[DT:G, :], pr[DT:G, :],
            op0=ALU.mult, op1=ALU.mult)

        # bounce B rows through DRAM to broadcast across 128 partitions
        # (C is consumed only via the transposed bcT copy)
        for q in range(NH):
            dma(bc_dram[ch, q, 0:NPH], bc_n[q * NPH:(q + 1) * NPH, :])

        bcq_list = []
        for q in range(NH):
            bcq = bcpool.tile([128, NPH * TC], BF16, tag=f"bcq{q}")
            nc.sync.dma_start(
                bcq[:].rearrange("p (j t) -> p j t", j=NPH),
                bc_dram[ch, q, 0:NPH].unsqueeze(0).broadcast_to(
                    (128, NPH, TC)))
            bcq_list.append(bcq)

        # ---- stage D: dt_proj + softplus (pair-fused exp/ln) -------------
        delta_all = dltpool.tile([128, NDT * TC], BF16, tag="dltall")
        u_all = upool.tile([128, NDT * TC], BF16, tag="uall")
        for pr_ in range(NPAIR):
            esp = esppool.tile([128, TC2], BF16, tag="esp")
            for dl in range(2):
                dt = pr_ * 2 + dl
                pd = pp_d.tile([128, TC], F32, tag="pd")
                nc.tensor.matmul(pd[:],
                                 _r(dt_wT[:, dt * 128:(dt + 1) * 128]),
                                 _r(delta_n[:]), start=True, stop=False)
                nc.tensor.matmul(pd[:],
                                 _r(b_dt[0:1, dt * 128:(dt + 1) * 128]),
                                 _r(ones[0:1, 0:TC]), start=False, stop=True)
                # softplus phase 1 (same table set as ln — no thrash)
                nc.scalar.activation(esp[:, dl * TC:(dl + 1) * TC], pd[:],
                                     AF.Exp)
            dlt = delta_all[:, pr_ * TC2:(pr_ + 1) * TC2]
            nc.scalar.activation(dlt, esp[:], AF.Ln, bias=1.0)
            up = u_all[:, pr_ * TC2:(pr_ + 1) * TC2]
            nc.vector.tensor_mul(up, dlt, h_all[:, pr_ * TC2:(pr_ + 1) * TC2])

        # ---- stage E: scans in n-quarter-groups + PE readout -------------
        # transpose B/C rows once: bcT[t', tblk, row] = bc_n[row, tblk*128+t']
        bcT = bctpool.tile([128, TBLK * 2 * N], BF16, tag="bcT")
        bcT3 = bcT[:].rearrange("p (j r) -> p j r", j=TBLK)
        nc.sync.dma_start_transpose(bcT3, bc_n[:])
        bcTf = bctpool.tile([128, TBLK * 2 * N], F32, tag="bcTf")
        nc.vector.tensor_copy(bcTf[:], bcT[:])

        y_all = ypool.tile([128, NDT * TC], BF16, tag="yall")
        y_all3 = y_all[:].rearrange("p (j t) -> p j t", j=NDT)
        for dhalf in range(2):
            yT_ps = [pp_y.tile([128, 512], F32, tag=f"yt{tb}",
                               name=f"yt{tb}_{ch}_{dhalf}")
                     for tb in range(TBLK)]
            for qg in range(NQ):
                # diag weights diag[t1,t2] = I*C[n,t1] via ACT copy-with-scale
                diag = dgpool.tile([128, NPQ * TBLK * 128], BF16, tag="dg")
                for nl in range(NPQ):
                    ng = qg * NPQ + nl
                    for tb in range(TBLK):
                        j = nl * TBLK + tb
                        nc.scalar.activation(
                            diag[:, j * 128:(j + 1) * 128], i_sb[:],
                            AF.Identity,
                            scale=bcTf[:, tb * 2 * N + N + ng:
                                       tb * 2 * N + N + ng + 1])
                # deltaA exps, pair-fused across dt
                dA_store = []
                for pl in range(2):
                    pr_ = dhalf * 2 + pl
                    dlt_pair3 = delta_all[
                        :, pr_ * TC2:(pr_ + 1) * TC2].rearrange(
                        "p (d t) -> p d t", d=2)
                    dA = dApool.tile([128, 2 * NPQ * TC], BF16, tag="dA")
                    dA4 = dA[:].rearrange("p (d n t) -> p d n t", d=2, n=NPQ)
                    if A_UNIFORM:
                        for nl in range(NPQ):
                            ng = qg * NPQ + nl
                            nc.scalar.activation(
                                dA4[:, :, nl, :], dlt_pair3, AF.Exp,
                                scale=A_sb[0][:, ng:ng + 1])
                    else:
                        for dl in range(2):
                            dt = pr_ * 2 + dl
                            for nl in range(NPQ):
                                ng = qg * NPQ + nl
                                nc.scalar.activation(
                                    dA4[:, dl, nl, :],
                                    delta_all[:, dt * TC:(dt + 1) * TC],
                                    AF.Exp, scale=A_sb[dt][:, ng:ng + 1])
                    dA_store.append(dA)
                hsT = hstpool.tile([128, NPQ * TBLK * 512], BF16, tag="hsT")
                hsT3 = hsT[:].rearrange("p (j q) -> p j q", j=NPQ * TBLK)
                for dl in range(4):
                    dt = dhalf * 4 + dl
                    u_t = u_all[:, dt * TC:(dt + 1) * TC]
                    dA = dA_store[dl // 2][
                        :, (dl % 2) * NPQ * TC:(dl % 2 + 1) * NPQ * TC]
                    dA3 = dA.rearrange("p (n t) -> p n t", n=NPQ)
                    bx = bxpool.tile([128, NPQ * TC], BF16, tag="bx")
                    bx3 = bx[:].rearrange("p (n t) -> p n t", n=NPQ)
                    u3 = u_t.unsqueeze(1).broadcast_to((128, NPQ, TC))
                    bcq = bcq_list[qg // 2]
                    bv = bcq[:, (qg % 2) * NPQ * TC:(qg % 2 + 1) * NPQ * TC]
                    nc.vector.tensor_mul(
                        bx3, u3, bv.rearrange("p (n t) -> p n t", n=NPQ))
                    ss = state[:, dt * N + qg * NPQ:dt * N + (qg + 1) * NPQ]
                    hs = hspool.tile([128, NPQ * TC], BF16, tag="hs")
                    if cb == 0:
                        nc.vector.memset(dA3[:, 1:NPQ, 0:1], 0.0)
                        nc.vector.tensor_tensor_scan(
                            hs[:, :], dA, bx[:, :], 0.0,
                            op0=ALU.mult, op1=ALU.add)
                    else:
                        a0 = trpool.tile([128, NPQ], F32, tag="a0")
                        nc.vector.tensor_copy(a0[:], dA3[:, :, 0])
                        nc.vector.memset(dA3[:, :, 0:1], 0.0)
                        nc.vector.tensor_mul(a0[:], a0[:], ss)
                        nc.vector.tensor_add(bx3[:, :, 0], bx3[:, :, 0], a0[:])
                        nc.vector.tensor_tensor_scan(
                            hs[:, :], dA, bx[:, :], 0.0,
                            op0=ALU.mult, op1=ALU.add)
                    if cb < CPB - 1:
                        hs3 = hs[:].rearrange("p (n t) -> p n t", n=NPQ)
                        nc.vector.tensor_copy(ss, hs3[:, :, TC - 1])
                    # block-transpose hs into the consolidated hsT tile
                    nc.sync.dma_start_transpose(
                        hsT3[:, :, dl * 128:(dl + 1) * 128], hs[:])
                # readout: yT[t',d] += sum_n C[n,t']*hsT[t',(n,tblk),d]
                for nl in range(NPQ):
                    for tb in range(TBLK):
                        j = nl * TBLK + tb
                        jd = nl * TBLK + tb
                        nc.tensor.matmul(
                            yT_ps[tb][:], _r(diag[:, jd * 128:(jd + 1) * 128]),
                            _r(hsT3[:, j, :]),
                            start=(qg == 0 and nl == 0),
                            stop=(qg == NQ - 1 and nl == NPQ - 1),
                            skip_group_check=True)
            # close this d-half: psum -> sbuf -> transpose back to (d, t)
            for tb in range(TBLK):
                ysb = ytpool.tile([128, 512], BF16, tag="ysb")
                nc.scalar.copy(ysb[:], yT_ps[tb][:])
                nc.sync.dma_start_transpose(
                    y_all3[:, dhalf * 4:(dhalf + 1) * 4,
                           tb * 128:(tb + 1) * 128], ysb[:])

        # D_skip: y += D * h
        y_list = []
        for dt in range(NDT):
            y_t = y_all[:, dt * TC:(dt + 1) * TC]
            nc.vector.scalar_tensor_tensor(
                y_t, h_list[dt], d_col[dt][:], y_t,
                op0=ALU.mult, op1=ALU.add)
            y_list.append(y_t)

        # ---- stage F: out_proj ------------------------------------------
        for tt in range(TC // 128):
            po = pp_o.tile([128, DM], F32, tag="po")
            for dt in range(NDT):
                nc.tensor.matmul(
                    po[:], _r(y_list[dt][:, tt * 128:(tt + 1) * 128]),
                    _r(out_wT[dt][:]), start=(dt == 0), stop=False)
            if FOLD_D_HOST:
                for dt in range(NDT):
                    nc.tensor.matmul(
                        po[:], _r(h_list[dt][:, tt * 128:(tt + 1) * 128]),
                        _r(out_wD[dt][:]), start=False, stop=False)
            nc.tensor.matmul(po[:], _r(ones[0:1, 0:128]), _r(b_out[0:1, :]),
                             start=False, stop=True)
            o_sb = opool.tile([128, DM], F32, tag="osb")
            nc.scalar.copy(o_sb[:], po[:])
            dma(outs["y_out"][bb, cb * TC + tt * 128:cb * TC + (tt + 1) * 128,
                              :], o_sb[:])

    st.close()


# ---------------------------------------------------------------- runner
_CACHE = {}


def _build_program(a_uniform):
    global A_UNIFORM
    key = ("nc", a_uniform)
    if key in _CACHE:
        return _CACHE[key]
    A_UNIFORM = a_uniform
    nc = bacc.Bacc("TRN2", target_bir_lowering=False, debug=False,
                   num_devices=NCORES)
    ins, outs = declare_ios(nc)
    with tile.TileContext(nc) as t:
        emit(t, outs, ins)
    nc.compile()
    _CACHE[key] = nc
    return nc


LAST_RESULT = None


def kernel(**inputs) -> np.ndarray:
    global LAST_RESULT
    import os
    from concourse.bass_utils import run_bass_kernel_spmd

    w = host_weights(inputs)
    # fused deltaA exps rely on A rows being identical across channels
    a_uniform = bool(np.allclose(w["A_neg"], w["A_neg"][0:1, :],
                                 rtol=0, atol=0))
    nc = _build_program(a_uniform)
    in_maps = []
    for c in range(NCORES):
        m = dict(w)
        m["xT"] = host_x_shard(inputs["x"], c)
        in_maps.append(m)
    trace = bool(os.environ.get("MIM_TRACE"))
    res = run_bass_kernel_spmd(nc, in_maps, list(range(NCORES)),
                               trace=trace)
    LAST_RESULT = res
    out = np.concatenate([res.results[c]["y_out"] for c in range(NCORES)],
                         axis=0)
    return out.astype(np.float32)
